# revision 1
# baseline (speedup 1.0000x reference)
"""2-layer LSTM (B=128, T=256, D=512, H=1024) + linear head + ELU on 8 trn2 cores.

Strategy (all hardcoded for this problem):
  - The chip has 2 dies x 4 cores; cross-die remote DMA doesn't deliver here,
    so the batch (128) is split across dies (64+64) - zero cross-die traffic.
  - Within a die: 4-way tensor-parallel. Each core owns a 256-wide slice of
    the hidden dim; per step it computes its 1024 local gate columns for its
    64-row batch half as a col-packed [128, 512] PSUM tile (partitions 0-63 =
    [i|o] cols, 64-127 = [f|g] cols), does the elementwise locally, PE-
    transposes its h slice, and remote-DMA-broadcasts it SBUF->SBUF to the 3
    die peers + itself (XOR-relative routing). Receiver slot k holds the h
    slice of core (self^k); weight rows are pre-permuted per core on the host
    to match, so no data reshuffling is ever needed on device.
  - xg0 = X @ W_ih0^T + b is precomputed per step (phase 1); xg1 is fused
    into the layer-0 recurrence (computed from the gathered h0 of the
    previous step), so layer-1 inputs never round-trip through a big matmul
    phase.
  - Everything is fp32; all weights stay SBUF-resident.
"""

import sys
from contextlib import ExitStack

import ml_dtypes
import numpy as np

for _p in ("/opt/trn_rl_repo", "/root/.axon_site/_ro/trn_rl_repo"):
    if _p not in sys.path:
        sys.path.append(_p)

import concourse.bacc as bacc
import concourse.mybir as mybir
import concourse.tile as tile
from concourse.bass_utils import run_bass_kernel_spmd
from concourse.masks import make_identity
from concourse.tile_rust import add_dep_helper

F32 = mybir.dt.float32
BF16 = mybir.dt.bfloat16
AF = mybir.ActivationFunctionType

P = 128
T = 256
D = 512
H = 1024
BR = 256
BL = 64    # batch rows per die
HL = 256   # hidden units per core
NL = 1024  # local gate columns per core (i|o|f|g, 256 each)
NUM_CORES = 8


def _build(nc, n_steps):
    """Emit the whole program. Returns list of (inst, sem, val) waits to patch
    after TileContext exit (remote-arrival waits the scheduler can't model)."""
    xt_in = nc.dram_tensor("XT", [D, T * BL], BF16, kind="ExternalInput").ap()
    w0_in = nc.dram_tensor("W0", [D, NL], BF16, kind="ExternalInput").ap()
    wh0_in = nc.dram_tensor("Wh0", [H, NL], BF16, kind="ExternalInput").ap()
    wx1_in = nc.dram_tensor("Wx1", [H, NL], BF16, kind="ExternalInput").ap()
    wh1_in = nc.dram_tensor("Wh1", [H, NL], BF16, kind="ExternalInput").ap()
    wbr_in = nc.dram_tensor("Wbr", [H, BR], BF16, kind="ExternalInput").ap()
    b0_in = nc.dram_tensor("b0p", [P, 512], F32, kind="ExternalInput").ap()
    b1_in = nc.dram_tensor("b1p", [P, 512], F32, kind="ExternalInput").ap()
    bbr_in = nc.dram_tensor("bbrp", [BL, BR], F32, kind="ExternalInput").ap()
    y_out = nc.dram_tensor("y", [BL, BR], F32, kind="ExternalOutput").ap()

    xg0_d = nc.dram_tensor("xg0_d", [T, P, 512], F32).ap()
    xg1_d = nc.dram_tensor("xg1_d", [T, P, 512], F32).ap()

    # SBUF-resident weights
    sW0 = nc.alloc_sbuf_tensor("sW0", [P, 4, NL], BF16).ap()
    sWh0 = nc.alloc_sbuf_tensor("sWh0", [P, 8, NL], BF16).ap()
    sWx1 = nc.alloc_sbuf_tensor("sWx1", [P, 8, NL], BF16).ap()
    sWh1 = nc.alloc_sbuf_tensor("sWh1", [P, 8, NL], BF16).ap()
    sWbr = nc.alloc_sbuf_tensor("sWbr", [P, 8, BR], BF16).ap()
    sB0 = nc.alloc_sbuf_tensor("sB0", [P, 512], F32).ap()
    sB1 = nc.alloc_sbuf_tensor("sB1", [P, 512], F32).ap()
    sBbr = nc.alloc_sbuf_tensor("sBbr", [BL, BR], F32).ap()
    ident = nc.alloc_sbuf_tensor("ident", [BL, BL], F32).ap()

    # recurrence state / comm buffers (stable addresses for remote writes)
    gath0 = [nc.alloc_sbuf_tensor(f"g0_{j}", [P, 512], BF16).ap() for j in range(2)]
    gath1 = [nc.alloc_sbuf_tensor(f"g1_{j}", [P, 512], BF16).ap() for j in range(2)]
    snd0 = [nc.alloc_sbuf_tensor(f"s0_{j}", [P, P], BF16).ap() for j in range(2)]
    snd1 = [nc.alloc_sbuf_tensor(f"s1_{j}", [P, P], BF16).ap() for j in range(2)]
    cst = [nc.alloc_sbuf_tensor(f"c{l}", [BL, HL], F32).ap() for l in range(2)]

    rsem = [nc.alloc_semaphore("rsem0"), nc.alloc_semaphore("rsem1")]
    lsem = nc.alloc_semaphore("lsem")

    patches = []

    with tile.TileContext(nc) as tc:
        barrier_nop = nc.gpsimd.nop(nofuse=True)

        # weight loads
        w0v = w0_in.rearrange("(k p) n -> k p n", p=P)
        for k in range(4):
            nc.sync.dma_start(out=sW0[:, k], in_=w0v[k])
        for sb, src in ((sWh0, wh0_in), (sWx1, wx1_in), (sWh1, wh1_in)):
            v = src.rearrange("(k p) n -> k p n", p=P)
            for k in range(8):
                nc.sync.dma_start(out=sb[:, k], in_=v[k])
        wbrv = wbr_in.rearrange("(k p) n -> k p n", p=P)
        for k in range(8):
            nc.sync.dma_start(out=sWbr[:, k], in_=wbrv[k])
        nc.sync.dma_start(out=sB0, in_=b0_in)
        nc.sync.dma_start(out=sB1, in_=b1_in)
        nc.sync.dma_start(out=sBbr, in_=bbr_in)
        make_identity(nc, ident)
        nc.vector.memset(cst[0], 0.0)
        nc.vector.memset(cst[1], 0.0)

        stack = ExitStack()
        psum_pool = stack.enter_context(tc.tile_pool(name="psum", bufs=4, space="PSUM"))
        pt_pool = stack.enter_context(tc.tile_pool(name="ptp", bufs=4, space="PSUM"))
        xt_pool = stack.enter_context(tc.tile_pool(name="xtp", bufs=4))
        xg_pool = stack.enter_context(tc.tile_pool(name="xgp", bufs=4))
        gt_pool = stack.enter_context(tc.tile_pool(name="gtp", bufs=2))
        ev_pool = stack.enter_context(tc.tile_pool(name="evp", bufs=3))
        tmp_pool = stack.enter_context(tc.tile_pool(name="tmp", bufs=4))
        hd_pool = stack.enter_context(tc.tile_pool(name="hdp", bufs=1))
        first_prep = [None]

        # ---------------- phase 1: xg0[t] = X_t @ W0 + b0 (col-packed) -------
        xtv = xt_in.rearrange("(k p) tb -> p k tb", p=P)
        for t in range(n_steps):
            xt = xt_pool.tile([P, 4, BL], BF16, name="xt")
            nc.sync.dma_start(out=xt[:, :, :], in_=xtv[:, :, BL * t : BL * (t + 1)])
            ps = psum_pool.tile([P, 512], F32, name="ps")
            for k in range(4):
                nc.tensor.matmul(ps[0:BL, :], xt[:, k], sW0[:, k, 0:512],
                                 start=(k == 0), stop=(k == 3), tile_position=(0, 0))
                nc.tensor.matmul(ps[BL:P, :], xt[:, k], sW0[:, k, 512:NL],
                                 start=(k == 0), stop=(k == 3), tile_position=(0, 64))
            ev = ev_pool.tile([P, 512], F32, name="ev")
            nc.vector.tensor_add(ev, ps, sB0)
            nc.sync.dma_start(out=xg0_d[t], in_=ev)

        # ---------------- recurrence (shared for both layers) ----------------
        def rec_step(layer, t, xg_src, gath, snd, sWh, c, rs, fuse_xg1):
            slot_base = 4 * layer
            if True:
                xg = xg_pool.tile([P, 512], F32, name="xg")
                nc.sync.dma_start(out=xg, in_=xg_src[t])
                if t == 0:
                    ps = psum_pool.tile([P, 512], F32, name="ps")
                    nc.vector.tensor_copy(ps, xg)
                    gates_src = ps
                    rec_first = None
                    rec_last = None
                else:
                    g = gath[(t - 1) % 2]
                    ps = psum_pool.tile([P, 512], F32, name="ps")
                    mms = []
                    for m in range(8):
                        mms.append(nc.tensor.matmul(
                            ps[0:BL, :], g[:, BL * m : BL * (m + 1)],
                            sWh[:, m, 0:512], start=(m == 0), stop=(m == 7),
                            tile_position=(0, 0)))
                        mms.append(nc.tensor.matmul(
                            ps[BL:P, :], g[:, BL * m : BL * (m + 1)],
                            sWh[:, m, 512:NL], start=(m == 0), stop=(m == 7),
                            tile_position=(0, 64)))
                    patches.append((mms[0], rs, 8 * t))
                    for mm in mms[1:]:
                        add_dep_helper(mm.ins, mms[0].ins, sync=False,
                                       reason="rec mms after gated first")
                    rec_first, rec_last = mms[0], mms[-1]
                    nc.vector.tensor_add(ps, ps, xg)
                    gates_src = ps

                # fused xg1 for step t-1 (layer 0 only), reading same gather buf
                if fuse_xg1 and t >= 1:
                    gprev = gath[(t - 1) % 2]
                    ps2 = psum_pool.tile([P, 512], F32, name="ps")
                    x1mms = []
                    for m in range(8):
                        x1mms.append(nc.tensor.matmul(
                            ps2[0:BL, :], gprev[:, BL * m : BL * (m + 1)],
                            sWx1[:, m, 0:512], start=(m == 0), stop=(m == 7),
                            tile_position=(0, 0)))
                        x1mms.append(nc.tensor.matmul(
                            ps2[BL:P, :], gprev[:, BL * m : BL * (m + 1)],
                            sWx1[:, m, 512:NL], start=(m == 0), stop=(m == 7),
                            tile_position=(0, 64)))
                    if rec_last is not None:
                        add_dep_helper(x1mms[0].ins, rec_last.ins, sync=False,
                                       reason="xg1 after rec")
                    else:
                        patches.append((x1mms[0], rs, 8 * t))
                    for mm in x1mms[1:]:
                        add_dep_helper(mm.ins, x1mms[0].ins, sync=False,
                                       reason="xg1 mms chain")
                    xv = ev_pool.tile([P, 512], F32, name="ev")
                    nc.vector.tensor_add(xv, ps2, sB1)
                    nc.sync.dma_start(out=xg1_d[t - 1], in_=xv)
                    xg1_last = x1mms[-1]
                else:
                    xg1_last = rec_last

                # elementwise: gates layout p0-63 = [i|o], p64-127 = [f|g]
                # (gates stay in PSUM at t>0, SBUF at t==0; same-space 2-input
                # DVE ops need equal base partitions, so tanh(g) bounces to an
                # SBUF tile at base 0 and f/o pair with SBUF operands.)
                gs = gates_src
                gsb = tmp_pool.tile([BL, HL], F32, name="gsb")
                nc.scalar.activation(gsb, gs[BL:P, 256:512], AF.Tanh)  # g -> @0 SB
                nc.scalar.activation(gs[BL:P, 0:256], gs[BL:P, 0:256], AF.Sigmoid)
                nc.scalar.activation(gs[0:BL, 0:512], gs[0:BL, 0:512], AF.Sigmoid)
                t2 = tmp_pool.tile([BL, HL], F32, name="t2")
                nc.vector.tensor_mul(t2, gs[BL:P, 0:256], c)          # f * c
                t1 = tmp_pool.tile([BL, HL], F32, name="t1")
                nc.vector.tensor_mul(t1, gs[0:BL, 0:256], gsb)        # i * g
                nc.vector.tensor_add(c, t1, t2)
                tcn = tmp_pool.tile([BL, HL], F32, name="tc")
                nc.scalar.activation(tcn, c, AF.Tanh)
                h = tmp_pool.tile([BL, HL], F32, name="h")
                nc.vector.tensor_mul(h, gs[0:BL, 256:512], tcn)       # o * tanh(c)

                # transpose h -> [256, 64] as two [128, 64] tiles, pack to send
                sndt = snd[t % 2]
                tr = []
                for j in range(2):
                    pt = pt_pool.tile([P, BL], F32, name="pt")
                    tr.append(nc.tensor.transpose(
                        pt[:, :], h[:, P * j : P * (j + 1)], ident[:, :]))
                    nc.vector.tensor_copy(sndt[:, BL * j : BL * (j + 1)], pt[:, :])
                if xg1_last is not None:
                    for tri in tr:
                        add_dep_helper(tri.ins, xg1_last.ins, sync=False,
                                       reason="transpose after xg1/rec")

                # broadcast h slice to die peers (XOR-relative, slots 0-3)
                gout = gath[t % 2]
                for k in range(4):
                    rd = [None] * 8
                    rd[slot_base + k] = (0, k)
                    prep = nc.gpsimd.remote_dma_broadcast(
                        gout[:, P * k : P * (k + 1)], sndt, rs, lsem, rdests=rd,
                        queue_num=layer)
                    if first_prep[0] is None:
                        first_prep[0] = prep
                        add_dep_helper(prep.ins, barrier_nop.ins, sync=False,
                                       reason="sends after entry barrier")
                nc.gpsimd.trigger_dma(count=None, queue_num=layer)

        for s in range(n_steps):
            rec_step(0, s, xg0_d, gath0, snd0, sWh0, cst[0], rsem[0], True)
            if s >= 1:
                rec_step(1, s - 1, xg1_d, gath1, snd1, sWh1, cst[1], rsem[1], False)
        # tail: xg1 for t = n_steps-1
        gprev = gath0[(n_steps - 1) % 2]
        ps2 = psum_pool.tile([P, 512], F32, name="ps")
        x1mms = []
        for m in range(8):
            x1mms.append(nc.tensor.matmul(
                ps2[0:BL, :], gprev[:, BL * m : BL * (m + 1)], sWx1[:, m, 0:512],
                start=(m == 0), stop=(m == 7), tile_position=(0, 0)))
            x1mms.append(nc.tensor.matmul(
                ps2[BL:P, :], gprev[:, BL * m : BL * (m + 1)], sWx1[:, m, 512:NL],
                start=(m == 0), stop=(m == 7), tile_position=(0, 64)))
        patches.append((x1mms[0], rsem[0], 8 * n_steps))
        for mm in x1mms[1:]:
            add_dep_helper(mm.ins, x1mms[0].ins, sync=False, reason="xg1 tail")
        xv = ev_pool.tile([P, 512], F32, name="ev")
        nc.vector.tensor_add(xv, ps2, sB1)
        nc.sync.dma_start(out=xg1_d[n_steps - 1], in_=xv)

        rec_step(1, n_steps - 1, xg1_d, gath1, snd1, sWh1, cst[1], rsem[1], False)

        # ---------------- head: ELU(h1_last @ Wbr + bbr) ---------------------
        glast = gath1[(n_steps - 1) % 2]
        psb = psum_pool.tile([P, 512], F32, name="ps")
        hmms = []
        for m in range(8):
            hmms.append(nc.tensor.matmul(
                psb[0:BL, 0:BR], glast[:, BL * m : BL * (m + 1)], sWbr[:, m, :],
                start=(m == 0), stop=(m == 7)))
        patches.append((hmms[0], rsem[1], 8 * n_steps))
        for mm in hmms[1:]:
            add_dep_helper(mm.ins, hmms[0].ins, sync=False, reason="head mms")
        br = hd_pool.tile([BL, BR], F32, name="br")
        nc.vector.tensor_add(br, psb[0:BL, 0:BR], sBbr)
        rl = hd_pool.tile([BL, BR], F32, name="rl")
        nc.scalar.activation(rl, br, AF.Relu)
        mn = hd_pool.tile([BL, BR], F32, name="mn")
        nc.vector.tensor_sub(mn, br, rl)
        ex = hd_pool.tile([BL, BR], F32, name="ex")
        nc.scalar.activation(ex, mn, AF.Exp)
        s1 = hd_pool.tile([BL, BR], F32, name="s1")
        nc.vector.tensor_add(s1, rl, ex)
        yv = hd_pool.tile([BL, BR], F32, name="yv")
        nc.vector.tensor_scalar_add(yv, s1, -1.0)
        nc.sync.dma_start(out=y_out, in_=yv)
        stack.close()

    # post-Tile patches (scheduler can't model remote increments)
    nc._bir_kernel_barrier_sem_replica_groups.append(set(range(NUM_CORES)))
    barrier_nop.wait_op(nc._bir_kernel_barrier_sem, nc.bir_kernel_barrier_sem_inc,
                        "sem-ge", check=False)
    for inst, sem, val in patches:
        if val > 0:
            inst.wait_op(sem, val, "sem-ge", check=False)
    return patches


def build_program(n_steps=T):
    nc = bacc.Bacc("TRN2", target_bir_lowering=False, debug=False,
                   num_devices=NUM_CORES, num_swdge_queues=2)
    _build(nc, n_steps)
    nc.compile()
    return nc


def prepare_inputs(X, W_ih0, W_hh0, b_ih0, b_hh0, W_ih1, W_hh1, b_ih1, b_hh1,
                   W_br, b_br, n_steps=T):
    """Host-side sharding + per-core weight permutation. Returns in_maps."""
    X = np.asarray(X, np.float32)
    in_maps = []
    for r in range(NUM_CORES):
        die, s = divmod(r, 4)
        bs = slice(BL * die, BL * (die + 1))
        # local gate columns: [i|o|f|g], each the core's 256-wide hidden slice
        hsl = np.arange(HL * s, HL * (s + 1))
        cols = np.concatenate([0 * H + hsl, 3 * H + hsl, 1 * H + hsl, 2 * H + hsl])
        # k-tile (128-row) order matching gather layout: slot k holds core s^k
        perm = np.concatenate(
            [np.arange(HL * (s ^ k), HL * (s ^ k) + HL) for k in range(4)])

        Xh = X[bs, :n_steps]                                   # [64, T, 512]
        XT = np.ascontiguousarray(
            Xh.transpose(2, 1, 0).reshape(D, n_steps * BL))    # [512, T*64]
        if n_steps < T:
            XT = np.concatenate(
                [XT, np.zeros((D, (T - n_steps) * BL), np.float32)], axis=1)

        W0 = W_ih0.T[:, cols]                                  # [512, 1024]
        Wh0 = W_hh0.T[perm][:, cols]                           # [1024, 1024]
        Wx1 = W_ih1.T[perm][:, cols]
        Wh1 = W_hh1.T[perm][:, cols]
        Wbr = W_br.T[perm]                                     # [1024, 256]
        b0 = (b_ih0 + b_hh0)[cols]
        b1 = (b_ih1 + b_hh1)[cols]
        b0p = np.concatenate([np.tile(b0[None, 0:512], (BL, 1)),
                              np.tile(b0[None, 512:NL], (BL, 1))], axis=0)
        b1p = np.concatenate([np.tile(b1[None, 0:512], (BL, 1)),
                              np.tile(b1[None, 512:NL], (BL, 1))], axis=0)
        bbrp = np.tile(np.asarray(b_br, np.float32)[None, :], (BL, 1))
        in_maps.append({
            "XT": XT.astype(np.float32).astype(ml_dtypes.bfloat16),
            "W0": np.ascontiguousarray(np.asarray(W0, np.float32).astype(ml_dtypes.bfloat16)),
            "Wh0": np.ascontiguousarray(np.asarray(Wh0, np.float32).astype(ml_dtypes.bfloat16)),
            "Wx1": np.ascontiguousarray(np.asarray(Wx1, np.float32).astype(ml_dtypes.bfloat16)),
            "Wh1": np.ascontiguousarray(np.asarray(Wh1, np.float32).astype(ml_dtypes.bfloat16)),
            "Wbr": np.ascontiguousarray(np.asarray(Wbr, np.float32).astype(ml_dtypes.bfloat16)),
            "b0p": np.ascontiguousarray(b0p, np.float32),
            "b1p": np.ascontiguousarray(b1p, np.float32),
            "bbrp": np.ascontiguousarray(bbrp, np.float32),
        })
    return in_maps


_cached_nc = None


def kernel(**inputs):
    global _cached_nc
    if _cached_nc is None:
        _cached_nc = build_program(T)
    in_maps = prepare_inputs(**inputs, n_steps=T)
    res = run_bass_kernel_spmd(_cached_nc, in_maps, list(range(NUM_CORES)))
    out = np.concatenate([res.results[0]["y"], res.results[4]["y"]], axis=0)
    return out.astype(np.float32)



# revision 9
# speedup vs baseline: 1.7916x; 1.7916x over previous
"""2-layer LSTM (B=128, T=256, D=512, H=1024) + linear head + ELU on 8 trn2 cores.

Strategy: time-phased pure data-parallel (zero inter-core communication).
  - Remote-DMA on this platform has ~7us/call latency (size-independent) and
    only tolerates one exact descriptor pattern; any per-step exchange is
    latency-doomed (measured: the TP4 baseline spent ~70% of its time waiting
    on remote h broadcasts). So: no comms at all.
  - Each core owns 16 batch rows and runs the FULL recurrence for them:
    phase 1 = layer-0 for all 256 steps (weights W0+Wh0 in SBUF, 12MB),
    h0_t^T spilled to DRAM; then weights are swapped in-place for
    Wx1+Wh1 (16MB) and phase 2 = layer-1 consumes h0T from DRAM; head at the
    end. All four weight matrices together (28.5MB) would NOT fit SBUF -
    the phase split is what makes DP possible.
  - Per step, gates are computed in a PACKED layout [128, 1024]: partition
    group 32g..32g+16 = gate g (i,f,g,o) for the 16 rows; 4 col-groups of the
    PE run concurrently via tile_position=(0,32g), so the 16-row matmuls still
    stream the full weight bandwidth. h_t is transposed back to [128,16]
    k-tiles on the PE for the next step's stationary operand.
"""

import sys
from contextlib import ExitStack

import ml_dtypes
import numpy as np

for _p in ("/opt/trn_rl_repo", "/root/.axon_site/_ro/trn_rl_repo"):
    if _p not in sys.path:
        sys.path.append(_p)

import concourse.bacc as bacc
import concourse.mybir as mybir
import concourse.tile as tile
from concourse.bass_utils import run_bass_kernel_spmd
from concourse.masks import make_identity

F32 = mybir.dt.float32
BF16 = mybir.dt.bfloat16
AF = mybir.ActivationFunctionType

P = 128
T = 256
D = 512
H = 1024
BR = 256
R = 16          # batch rows per core
NUM_CORES = 8


def _build(nc, n_steps):
    xt_in = nc.dram_tensor("XT", [4, P, T * R], BF16, kind="ExternalInput").ap()
    w1_in = nc.dram_tensor("W1", [12, P, 4 * H], BF16, kind="ExternalInput").ap()
    w2_in = nc.dram_tensor("W2", [16, P, 4 * H], BF16, kind="ExternalInput").ap()
    wbr_in = nc.dram_tensor("Wbr", [8, P, BR], BF16, kind="ExternalInput").ap()
    b0_in = nc.dram_tensor("b0p", [P, H], F32, kind="ExternalInput").ap()
    b1_in = nc.dram_tensor("b1p", [P, H], F32, kind="ExternalInput").ap()
    bbr_in = nc.dram_tensor("bbrp", [R, BR], F32, kind="ExternalInput").ap()
    y_out = nc.dram_tensor("y", [R, BR], F32, kind="ExternalOutput").ap()
    h0t_d = nc.dram_tensor("h0t_d", [T, P, P], BF16).ap()

    wslot = nc.alloc_sbuf_tensor("wslot", [P, 16, 4 * H], BF16).ap()
    sXT = nc.alloc_sbuf_tensor("sXT", [P, 4, T * R], BF16).ap()
    sb0 = nc.alloc_sbuf_tensor("sb0", [P, H], F32).ap()
    sb1 = nc.alloc_sbuf_tensor("sb1", [P, H], F32).ap()
    sWbr = nc.alloc_sbuf_tensor("sWbr", [P, 8, BR], BF16).ap()
    sbbr = nc.alloc_sbuf_tensor("sbbr", [R, BR], F32).ap()
    ident = nc.alloc_sbuf_tensor("ident", [R, R], F32).ap()
    ring = [nc.alloc_sbuf_tensor(f"ring{j}", [P, P], BF16).ap() for j in range(2)]
    cst = nc.alloc_sbuf_tensor("cst", [R, H], F32).ap()
    tg = nc.alloc_sbuf_tensor("tg", [R, H], F32).ap()
    t1 = nc.alloc_sbuf_tensor("t1", [R, H], F32).ap()
    t2 = nc.alloc_sbuf_tensor("t2", [R, H], F32).ap()
    tcn = nc.alloc_sbuf_tensor("tcn", [R, H], F32).ap()
    hbuf = nc.alloc_sbuf_tensor("hbuf", [R, H], F32).ap()

    with tile.TileContext(nc) as tc:
        for k in range(4):
            nc.sync.dma_start(out=sXT[:, k], in_=xt_in[k])
        for s in range(12):
            nc.sync.dma_start(out=wslot[:, s], in_=w1_in[s])
        nc.sync.dma_start(out=sb0, in_=b0_in)
        nc.sync.dma_start(out=sb1, in_=b1_in)
        for j in range(8):
            nc.sync.dma_start(out=sWbr[:, j], in_=wbr_in[j])
        nc.sync.dma_start(out=sbbr, in_=bbr_in)
        make_identity(nc, ident)
        nc.vector.memset(ring[1], 0.0)
        nc.vector.memset(cst, 0.0)

        stack = ExitStack()
        psum_pool = stack.enter_context(tc.tile_pool(name="ps", bufs=2, space="PSUM"))
        pt_pool = stack.enter_context(tc.tile_pool(name="pt", bufs=2, space="PSUM"))
        h0_pool = stack.enter_context(tc.tile_pool(name="h0l", bufs=6))

        def step(t, n_xg, xg_stat, rec_slot0, sb, store_h0t):
            """One recurrence step. xg_stat(kt) -> stationary AP [128, 16]."""
            ps = psum_pool.tile([P, H], F32, name="ps")
            ringprev = ring[(t - 1) % 2]

            # xg matmuls (start accumulation), then recurrent matmuls
            def mm_group(kt_range, stat_fn, slot0, start_group, stop_group):
                n = len(kt_range)
                for i, kt in enumerate(kt_range):
                    stat = stat_fn(kt)
                    for g in range(4):
                        for c in range(2):
                            nc.tensor.matmul(
                                ps[32 * g : 32 * g + R, 512 * c : 512 * (c + 1)],
                                stat,
                                wslot[:, slot0 + kt,
                                      H * g + 512 * c : H * g + 512 * (c + 1)],
                                start=start_group and (i == 0),
                                stop=stop_group and (i == n - 1),
                                tile_position=(0, 32 * g),
                            )

            mm_group(range(n_xg), xg_stat, 0, True, False)
            mm_group(range(8), lambda kt: ringprev[:, R * kt : R * (kt + 1)],
                     rec_slot0, False, True)

            # gates += bias, then elementwise. packed: i@0, f@32, g@64, o@96
            nc.vector.tensor_add(ps, ps, sb)
            nc.scalar.activation(tg, ps[64 : 64 + R], AF.Tanh)
            nc.scalar.activation(ps[0:R], ps[0:R], AF.Sigmoid)
            nc.scalar.activation(ps[32 : 32 + R], ps[32 : 32 + R], AF.Sigmoid)
            nc.scalar.activation(ps[96 : 96 + R], ps[96 : 96 + R], AF.Sigmoid)
            nc.vector.tensor_mul(t2, ps[32 : 32 + R], cst)          # f * c
            nc.vector.tensor_mul(t1, ps[0:R], tg)                   # i * g
            nc.vector.tensor_add(cst, t1, t2)
            nc.scalar.activation(tcn, cst, AF.Tanh)
            h = hbuf
            nc.vector.tensor_mul(h, ps[96 : 96 + R], tcn)           # o * tanh(c)

            # transpose h -> [128, 16] k-tiles into ring[t%2]
            rt = ring[t % 2]
            pt = pt_pool.tile([P, P], F32, name="pt")
            for k in range(8):
                nc.tensor.transpose(pt[:, R * k : R * (k + 1)],
                                    h[:, P * k : P * (k + 1)], ident[:, :])
                nc.vector.tensor_copy(rt[:, R * k : R * (k + 1)],
                                      pt[:, R * k : R * (k + 1)])
            if store_h0t:
                nc.sync.dma_start(out=h0t_d[t], in_=rt)

        # ---------------- phase 1: layer 0 ----------------
        for t in range(n_steps):
            step(t, 4, lambda kt, _t=t: sXT[:, kt, R * _t : R * (_t + 1)],
                 4, sb0, True)

        # ---------------- weight swap + state reset -------
        for s in range(16):
            nc.sync.dma_start(out=wslot[:, s], in_=w2_in[s])
        nc.vector.memset(ring[1], 0.0)
        nc.vector.memset(cst, 0.0)

        # ---------------- phase 2: layer 1 ----------------
        for t in range(n_steps):
            h0in = h0_pool.tile([P, P], BF16, name="h0in")
            nc.sync.dma_start(out=h0in, in_=h0t_d[t])
            step(t, 8, lambda kt, _h=h0in: _h[:, R * kt : R * (kt + 1)],
                 8, sb1, False)

        # ---------------- head: ELU(h1_last @ Wbr + bbr) --
        glast = ring[(n_steps - 1) % 2]
        psh = psum_pool.tile([P, H], F32, name="ps")
        for k in range(8):
            nc.tensor.matmul(psh[0:R, 0:BR], glast[:, R * k : R * (k + 1)],
                             sWbr[:, k], start=(k == 0), stop=(k == 7),
                             tile_position=(0, 0))
        br = t1[:, 0:BR]
        nc.vector.tensor_add(br, psh[0:R, 0:BR], sbbr)
        rl = t2[:, 0:BR]
        nc.scalar.activation(rl, br, AF.Relu)
        mn = tg[:, 0:BR]
        nc.vector.tensor_sub(mn, br, rl)
        ex = tcn[:, 0:BR]
        nc.scalar.activation(ex, mn, AF.Exp)
        s1 = t1[:, BR : 2 * BR]
        nc.vector.tensor_add(s1, rl, ex)
        yv = hbuf[:, 0:BR]
        nc.vector.tensor_scalar_add(yv, s1, -1.0)
        nc.sync.dma_start(out=y_out, in_=yv)
        stack.close()


def build_program(n_steps=T):
    nc = bacc.Bacc("TRN2", target_bir_lowering=False, debug=False,
                   num_devices=NUM_CORES)
    _build(nc, n_steps)
    nc.compile()
    return nc


def _bf(a):
    return np.ascontiguousarray(np.asarray(a, np.float32).astype(ml_dtypes.bfloat16))


def prepare_inputs(X, W_ih0, W_hh0, b_ih0, b_hh0, W_ih1, W_hh1, b_ih1, b_hh1,
                   W_br, b_br, n_steps=T):
    X = np.asarray(X, np.float32)
    W0t = np.asarray(W_ih0, np.float32).T.reshape(4, P, 4 * H)
    Wh0t = np.asarray(W_hh0, np.float32).T.reshape(8, P, 4 * H)
    Wx1t = np.asarray(W_ih1, np.float32).T.reshape(8, P, 4 * H)
    Wh1t = np.asarray(W_hh1, np.float32).T.reshape(8, P, 4 * H)
    Wbrt = np.asarray(W_br, np.float32).T.reshape(8, P, BR)
    w1 = _bf(np.concatenate([W0t, Wh0t], axis=0))
    w2 = _bf(np.concatenate([Wx1t, Wh1t], axis=0))
    wbr = _bf(Wbrt)

    def packed_bias(b):
        out = np.zeros((P, H), np.float32)
        for g in range(4):
            out[32 * g : 32 * g + R, :] = b[H * g : H * (g + 1)][None, :]
        return out

    b0p = packed_bias(np.asarray(b_ih0, np.float32) + np.asarray(b_hh0, np.float32))
    b1p = packed_bias(np.asarray(b_ih1, np.float32) + np.asarray(b_hh1, np.float32))
    bbrp = np.ascontiguousarray(
        np.tile(np.asarray(b_br, np.float32)[None, :], (R, 1)))

    in_maps = []
    for r in range(NUM_CORES):
        Xr = X[R * r : R * (r + 1), :n_steps]          # [16, n, 512]
        XT = Xr.transpose(2, 1, 0).reshape(D, n_steps * R)   # [512, n*16]
        if n_steps < T:
            XT = np.concatenate(
                [XT, np.zeros((D, (T - n_steps) * R), np.float32)], axis=1)
        in_maps.append({
            "XT": _bf(XT.reshape(4, P, T * R)),
            "W1": w1,
            "W2": w2,
            "Wbr": wbr,
            "b0p": np.ascontiguousarray(b0p),
            "b1p": np.ascontiguousarray(b1p),
            "bbrp": bbrp,
        })
    return in_maps


_cached_nc = None


def kernel(**inputs):
    global _cached_nc
    if _cached_nc is None:
        _cached_nc = build_program(T)
    in_maps = prepare_inputs(**inputs, n_steps=T)
    res = run_bass_kernel_spmd(_cached_nc, in_maps, list(range(NUM_CORES)))
    out = np.concatenate([res.results[r]["y"] for r in range(NUM_CORES)], axis=0)
    return out.astype(np.float32)


# revision 20
# speedup vs baseline: 3.1385x; 1.7518x over previous
"""2-layer LSTM (B=128, T=256, D=512, H=1024) + linear head + ELU on 8 trn2 cores.

Strategy: time-phased pure data-parallel (zero inter-core communication).
  - Remote-DMA on this platform has ~7us/call latency (size-independent) and
    only tolerates one exact descriptor pattern; any per-step exchange is
    latency-doomed (measured: the TP4 baseline spent ~70% of its time waiting
    on remote h broadcasts). So: no comms at all.
  - Each core owns 16 batch rows and runs the FULL recurrence for them:
    phase 1 = layer-0 for all 256 steps (weights W0+Wh0 in SBUF, 12MB),
    h0_t^T spilled to DRAM; then weights are swapped in-place for
    Wx1+Wh1 (16MB) and phase 2 = layer-1 consumes h0T from DRAM; head at the
    end. All four weight matrices together (28.5MB) would NOT fit SBUF -
    the phase split is what makes DP possible.
  - Per step, gates are computed in a PACKED layout [128, 1024]: partition
    group 32g..32g+16 = gate g (i,f,g,o) for the 16 rows; 4 col-groups of the
    PE run concurrently via tile_position=(0,32g), so the 16-row matmuls still
    stream the full weight bandwidth. h_t is transposed back to [128,16]
    k-tiles on the PE for the next step's stationary operand.
"""

import sys
from contextlib import ExitStack

import ml_dtypes
import numpy as np

for _p in ("/opt/trn_rl_repo", "/root/.axon_site/_ro/trn_rl_repo"):
    if _p not in sys.path:
        sys.path.append(_p)

import concourse.bacc as bacc
import concourse.mybir as mybir
import concourse.tile as tile
from concourse.bass_utils import run_bass_kernel_spmd
from concourse.masks import make_identity

F32 = mybir.dt.float32
BF16 = mybir.dt.bfloat16
AF = mybir.ActivationFunctionType

P = 128
T = 256
D = 512
H = 1024
BR = 256
R = 16          # batch rows per core
NUM_CORES = 8


def _build(nc, n_steps):
    xt_in = nc.dram_tensor("XT", [4, P, T * R], BF16, kind="ExternalInput").ap()
    w1_in = nc.dram_tensor("W1", [12, P, 4 * H], BF16, kind="ExternalInput").ap()
    w2_in = nc.dram_tensor("W2", [16, P, 4 * H], BF16, kind="ExternalInput").ap()
    wbr_in = nc.dram_tensor("Wbr", [8, P, BR], BF16, kind="ExternalInput").ap()
    b0_in = nc.dram_tensor("b0p", [P, H], F32, kind="ExternalInput").ap()
    b1_in = nc.dram_tensor("b1p", [P, H], F32, kind="ExternalInput").ap()
    bbr_in = nc.dram_tensor("bbrp", [R, BR], F32, kind="ExternalInput").ap()
    y_out = nc.dram_tensor("y", [R, BR], F32, kind="ExternalOutput").ap()
    h0t_d = nc.dram_tensor("h0t_d", [T, P, 2 * P], BF16).ap()

    wslot = nc.alloc_sbuf_tensor("wslot", [P, 16, 4 * H], BF16).ap()
    sXT = nc.alloc_sbuf_tensor("sXT", [P, 4, T * R], BF16).ap()
    sb0 = nc.alloc_sbuf_tensor("sb0", [P, H], F32).ap()
    sb1 = nc.alloc_sbuf_tensor("sb1", [P, H], F32).ap()
    sWbr = nc.alloc_sbuf_tensor("sWbr", [P, 8, BR], BF16).ap()
    sbbr = nc.alloc_sbuf_tensor("sbbr", [R, BR], F32).ap()
    ident = nc.alloc_sbuf_tensor("ident", [P, P], F32).ap()
    ring = [nc.alloc_sbuf_tensor(f"ring{j}", [P, 2 * P], BF16).ap() for j in range(2)]
    cst = nc.alloc_sbuf_tensor("cst", [P, 256], F32).ap()
    tg = nc.alloc_sbuf_tensor("tg", [P, 256], F32).ap()
    t1 = nc.alloc_sbuf_tensor("t1", [P, 256], F32).ap()
    t2 = nc.alloc_sbuf_tensor("t2", [P, 256], F32).ap()
    tcn = nc.alloc_sbuf_tensor("tcn", [P, 256], F32).ap()
    hbuf = nc.alloc_sbuf_tensor("hbuf", [P, 256], F32).ap()

    with tile.TileContext(nc) as tc:
        for k in range(4):
            nc.sync.dma_start(out=sXT[:, k], in_=xt_in[k])
        for s in range(12):
            nc.sync.dma_start(out=wslot[:, s], in_=w1_in[s])
        nc.sync.dma_start(out=sb0, in_=b0_in)
        nc.sync.dma_start(out=sb1, in_=b1_in)
        for j in range(8):
            nc.sync.dma_start(out=sWbr[:, j], in_=wbr_in[j])
        nc.sync.dma_start(out=sbbr, in_=bbr_in)
        make_identity(nc, ident)
        nc.vector.memset(ring[1], 0.0)
        nc.vector.memset(cst, 0.0)

        stack = ExitStack()
        psum_pool = stack.enter_context(tc.tile_pool(name="ps", bufs=2, space="PSUM"))
        pt_pool = stack.enter_context(tc.tile_pool(name="pt", bufs=2, space="PSUM"))
        h0_pool = stack.enter_context(tc.tile_pool(name="h0l", bufs=6))

        def stat_ap(buf, l):
            # stationary [128, 16] for logical k-tile l in a ring/h0in buffer
            return buf[:, P * (l % 2) + 32 * (l // 2) : P * (l % 2) + 32 * (l // 2) + R]

        def step(t, n_xg, xg_stat, rec_slot0, sb, store_h0t):
            """One recurrence step, chunk-stacked layout.

            PSUM gates [128, 1024]: partition 32j+b = batch row b, hidden
            chunk j (256-wide); column 256g+d = gate g (i,f,o,g' order),
            hidden offset d. Every eltwise op then runs on all 128 lanes.
            """
            ps = psum_pool.tile([P, H], F32, name="ps")
            ringprev = ring[(t - 1) % 2]

            def mm_group(kt_range, stat_fn, slot0, start_group, stop_group):
                n = len(kt_range)
                for i, kt in enumerate(kt_range):
                    stat = stat_fn(kt)
                    for j in range(4):
                        for c in range(2):
                            nc.tensor.matmul(
                                ps[32 * j : 32 * j + R, 512 * c : 512 * (c + 1)],
                                stat,
                                wslot[:, slot0 + kt,
                                      H * j + 512 * c : H * j + 512 * (c + 1)],
                                start=start_group and (i == 0),
                                stop=stop_group and (i == n - 1),
                                tile_position=(0, 32 * j),
                            )

            mm_group(range(n_xg), xg_stat, 0, True, False)
            mm_group(range(8), lambda kt: stat_ap(ringprev, kt),
                     rec_slot0, False, True)

            # gates += bias; cols: i=[0:256) f=[256:512) o=[512:768) g=[768:1024)
            nc.vector.tensor_add(ps, ps, sb)
            nc.scalar.activation(ps[:, 0:768], ps[:, 0:768], AF.Sigmoid)
            nc.scalar.activation(tg, ps[:, 768:1024], AF.Tanh)
            nc.vector.tensor_mul(t2, ps[:, 256:512], cst)           # f * c
            nc.vector.tensor_mul(t1, ps[:, 0:256], tg)              # i * g
            nc.vector.tensor_add(cst, t1, t2)
            nc.scalar.activation(tcn, cst, AF.Tanh)
            h = hbuf
            nc.vector.tensor_mul(h, ps[:, 512:768], tcn)            # o * tanh(c)

            # two full 128x128 transposes: h[:, 128c:+128] -> k-tiles 2j+c
            rt = ring[t % 2]
            for c in range(2):
                pt = pt_pool.tile([P, P], F32, name="pt")
                nc.tensor.transpose(pt[:, :], h[:, P * c : P * (c + 1)],
                                    ident[:, :])
                nc.vector.tensor_copy(rt[:, P * c : P * (c + 1)], pt[:, :])
            if store_h0t:
                nc.sync.dma_start(out=h0t_d[t], in_=rt)

        # ---------------- phase 1: layer 0 ----------------
        for t in range(n_steps):
            step(t, 4, lambda kt, _t=t: sXT[:, kt, R * _t : R * (_t + 1)],
                 4, sb0, True)

        # ---------------- weight swap + state reset -------
        for s in range(16):
            nc.sync.dma_start(out=wslot[:, s], in_=w2_in[s])
        nc.vector.memset(ring[1], 0.0)
        nc.vector.memset(cst, 0.0)

        # ---------------- phase 2: layer 1 ----------------
        for t in range(n_steps):
            h0in = h0_pool.tile([P, 2 * P], BF16, name="h0in")
            nc.sync.dma_start(out=h0in, in_=h0t_d[t])
            step(t, 8, lambda kt, _h=h0in: stat_ap(_h, kt), 8, sb1, False)

        # ---------------- head: ELU(h1_last @ Wbr + bbr) --
        glast = ring[(n_steps - 1) % 2]
        psh = psum_pool.tile([P, H], F32, name="ps")
        for k in range(8):
            nc.tensor.matmul(psh[0:R, 0:BR], stat_ap(glast, k),
                             sWbr[:, k], start=(k == 0), stop=(k == 7),
                             tile_position=(0, 0))
        br = t1[0:R, 0:BR]
        nc.vector.tensor_add(br, psh[0:R, 0:BR], sbbr)
        rl = t2[0:R, 0:BR]
        nc.scalar.activation(rl, br, AF.Relu)
        mn = tg[0:R, 0:BR]
        nc.vector.tensor_sub(mn, br, rl)
        ex = tcn[0:R, 0:BR]
        nc.scalar.activation(ex, mn, AF.Exp)
        s1 = cst[0:R, 0:BR]
        nc.vector.tensor_add(s1, rl, ex)
        yv = hbuf[0:R, 0:BR]
        nc.vector.tensor_scalar_add(yv, s1, -1.0)
        nc.sync.dma_start(out=y_out, in_=yv)
        stack.close()


def build_program(n_steps=T):
    nc = bacc.Bacc("TRN2", target_bir_lowering=False, debug=False,
                   num_devices=NUM_CORES)
    _build(nc, n_steps)
    nc.compile()
    return nc


def _bf(a):
    return np.ascontiguousarray(np.asarray(a, np.float32).astype(ml_dtypes.bfloat16))


def prepare_inputs(X, W_ih0, W_hh0, b_ih0, b_hh0, W_ih1, W_hh1, b_ih1, b_hh1,
                   W_br, b_br, n_steps=T):
    X = np.asarray(X, np.float32)

    def chunk_pack(wt):
        # pytorch cols [i|f|g|o] (1024 each) -> chunk-stacked layout:
        # new col 1024*j + 256*g + d = wt[:, 1024*G(g) + 256*j + d],
        # G = (i,f,o,g). Chunk j lands in PE col-group j; gate g in
        # column range [256g, 256g+256) of the packed psum.
        w4 = wt.reshape(-1, 4, 4, 256)[:, (0, 1, 3, 2)]   # [K, g, j, d]
        return np.transpose(w4, (0, 2, 1, 3)).reshape(-1, 4 * H)

    W0t = chunk_pack(np.asarray(W_ih0, np.float32).T).reshape(4, P, 4 * H)
    Wh0t = chunk_pack(np.asarray(W_hh0, np.float32).T).reshape(8, P, 4 * H)
    Wx1t = chunk_pack(np.asarray(W_ih1, np.float32).T).reshape(8, P, 4 * H)
    Wh1t = chunk_pack(np.asarray(W_hh1, np.float32).T).reshape(8, P, 4 * H)
    Wbrt = np.asarray(W_br, np.float32).T.reshape(8, P, BR)
    w1 = _bf(np.concatenate([W0t, Wh0t], axis=0))
    w2 = _bf(np.concatenate([Wx1t, Wh1t], axis=0))
    wbr = _bf(Wbrt)

    def packed_bias(b):
        arr = np.asarray(b, np.float32).reshape(4, 4, 256)[(0, 1, 3, 2), :, :]
        out = np.zeros((P, H), np.float32)
        for j in range(4):
            for g in range(4):
                out[32 * j : 32 * j + R, 256 * g : 256 * (g + 1)] = arr[g, j][None, :]
        return out

    b0p = packed_bias(np.asarray(b_ih0, np.float32) + np.asarray(b_hh0, np.float32))
    b1p = packed_bias(np.asarray(b_ih1, np.float32) + np.asarray(b_hh1, np.float32))
    bbrp = np.ascontiguousarray(
        np.tile(np.asarray(b_br, np.float32)[None, :], (R, 1)))

    in_maps = []
    for r in range(NUM_CORES):
        Xr = X[R * r : R * (r + 1), :n_steps]          # [16, n, 512]
        XT = Xr.transpose(2, 1, 0).reshape(D, n_steps * R)   # [512, n*16]
        if n_steps < T:
            XT = np.concatenate(
                [XT, np.zeros((D, (T - n_steps) * R), np.float32)], axis=1)
        in_maps.append({
            "XT": _bf(XT.reshape(4, P, T * R)),
            "W1": w1,
            "W2": w2,
            "Wbr": wbr,
            "b0p": np.ascontiguousarray(b0p),
            "b1p": np.ascontiguousarray(b1p),
            "bbrp": bbrp,
        })
    return in_maps


_cached_nc = None


def kernel(**inputs):
    global _cached_nc
    if _cached_nc is None:
        _cached_nc = build_program(T)
    in_maps = prepare_inputs(**inputs, n_steps=T)
    res = run_bass_kernel_spmd(_cached_nc, in_maps, list(range(NUM_CORES)))
    out = np.concatenate([res.results[r]["y"] for r in range(NUM_CORES)], axis=0)
    return out.astype(np.float32)
